# revision 1
# baseline (speedup 1.0000x reference)
"""Trainium2 Bass kernel for ConservativeGSAAttention.

Sharding: 8 cores = 4 batches x 2 head-groups (8 heads each).
Each core computes qkv-proj + attention + its half of c_proj for one batch;
the host sums the two partial c_proj outputs per batch (the "all-reduce").

Layout strategy (per core):
  - hidden_states passed transposed: hsT [E, T].
  - qkv proj computed in transposed layout: qT/kT [feat, token] (feat on
    partitions) so that scores matmuls need no transposes; v computed in
    [token, feat] layout for the AV matmul.
  - scores computed transposed: scoresT [keys, queries] (keys on partitions),
    block [128k x 512q]; fully-masked blocks are skipped; exp on ScalarE with
    the per-head splat scale/bias fused in; triangular 128x128 mask applied
    multiplicatively after exp on the diagonal blocks only.
  - softmax denominator comes for free from an appended ones-column in v
    (AV matmul row 64 = sum_k exp); normalization multiplies the AV output
    (64 rows) instead of the attention matrix (2048 rows).
"""

import math

import numpy as np

import concourse.bass as bass
import concourse.tile as tile
from concourse import bacc
from concourse import mybir
from concourse.bass_utils import run_bass_kernel_spmd

B, T, E, H, D = 4, 2048, 1024, 16, 64
HG = 8              # heads per core
F = HG * D          # 512 feats per group (for each of q, k, v)
P = 128
KT = E // P         # 8 contraction tiles for projections
TT = T // P         # 16 token tiles
QB = 512            # query block width
NQB = T // QB       # 4
FT = F // P         # 4 feat tiles per q/k/v group
FT_QK = 2 * FT      # 8 (q tiles then k tiles)
EB = 512            # c_proj output block width
NEB = E // EB       # 2

f32 = mybir.dt.float32
f32r = mybir.dt.float32r

AF = mybir.ActivationFunctionType


def _r(ap):
    """fp32r view of an fp32 AP for full-rate PE matmuls."""
    return ap.bitcast(f32r)


def build_program():
    nc = bacc.Bacc("TRN2", target_bir_lowering=False, debug=False)

    # ---- I/O ----
    hsT = nc.dram_tensor("hsT", [E, T], f32, kind="ExternalInput").ap()
    wqkT = nc.dram_tensor("wqkT", [E, 2 * F], f32, kind="ExternalInput").ap()
    wvT = nc.dram_tensor("wvT", [E, F], f32, kind="ExternalInput").ap()
    qk_bias = nc.dram_tensor("qk_bias", [P, FT_QK], f32, kind="ExternalInput").ap()
    v_bias = nc.dram_tensor("v_bias", [1, F], f32, kind="ExternalInput").ap()
    wpT = nc.dram_tensor("wpT", [F, E], f32, kind="ExternalInput").ap()
    bp_half = nc.dram_tensor("bp_half", [1, E], f32, kind="ExternalInput").ap()
    tri = nc.dram_tensor("tri", [P, P], f32, kind="ExternalInput").ap()
    act_s = nc.dram_tensor("act_s", [P, HG], f32, kind="ExternalInput").ap()
    act_b = nc.dram_tensor("act_b", [P, HG], f32, kind="ExternalInput").ap()
    out = nc.dram_tensor("out", [TT, P, E], f32, kind="ExternalOutput").ap()

    # ---- DRAM scratch ----
    qT_dr = nc.dram_tensor("qT_dr", [FT, P, T], f32).ap()
    kT_dr = nc.dram_tensor("kT_dr", [FT, P, T], f32).ap()
    v_dr = nc.dram_tensor("v_dr", [HG, TT, P, D + 1], f32).ap()
    ao_dr = nc.dram_tensor("ao_dr", [FT, P, T], f32).ap()
    rc_dr = nc.dram_tensor("rc_dr", [HG, NQB, 1, QB], f32).ap()

    from contextlib import ExitStack
    with tile.TileContext(nc) as tc, ExitStack() as ctx:
        def pool(name, bufs, space="SBUF"):
            return ctx.enter_context(tc.tile_pool(name=name, bufs=bufs, space=space))

        consts = pool("consts", 1)
        big = pool("big", 2)
        hs_pool = pool("hs", 2)
        stage = pool("stage", 4)
        vstage = pool("vstage", 2)
        vh_pool = pool("vh", 2)
        kh_pool = pool("kh", 2)
        q_pool = pool("q", 3)
        ao_pool = pool("aostage", 2)
        bc_pool = pool("bcast", 2)
        rc_pool = pool("rc", 2)
        aol_pool = pool("aol", 8)
        out_pool = pool("outp", 2)
        raw_pool = pool("raw", 4)
        msk_pool = pool("msk", 4)
        mm_ps = pool("mm_ps", 2, "PSUM")
        sc_ps = pool("sc_ps", 3, "PSUM")
        av_ps = pool("av_ps", 2, "PSUM")

        if True:
            # ---- resident constants ----
            wqk_sb = big.tile([P, KT, 2 * F], f32r, tag="big")
            nc.sync.dma_start(out=wqk_sb, in_=wqkT.bitcast(f32r).rearrange("(kt p) f -> p kt f", p=P))
            wv_sb = consts.tile([P, KT, F], f32r)
            nc.sync.dma_start(out=wv_sb, in_=wvT.bitcast(f32r).rearrange("(kt p) f -> p kt f", p=P))
            wp_sb = consts.tile([P, FT, E], f32r)
            nc.sync.dma_start(out=wp_sb, in_=wpT.bitcast(f32r).rearrange("(ft p) e -> p ft e", p=P))
            qkb_sb = consts.tile([P, FT_QK], f32)
            nc.sync.dma_start(out=qkb_sb, in_=qk_bias)
            vb_sb = consts.tile([P, 1, F], f32)
            nc.sync.dma_start(out=vb_sb, in_=v_bias.partition_broadcast(P))
            bp_sb = consts.tile([P, 1, E], f32)
            nc.sync.dma_start(out=bp_sb, in_=bp_half.partition_broadcast(P))
            tri_sb = consts.tile([P, P], f32)
            nc.sync.dma_start(out=tri_sb, in_=tri)
            acts_sb = consts.tile([P, HG], f32)
            nc.sync.dma_start(out=acts_sb, in_=act_s)
            actb_sb = consts.tile([P, HG], f32)
            nc.sync.dma_start(out=actb_sb, in_=act_b)
            ones_col = consts.tile([P, 1], f32)
            nc.vector.memset(ones_col, 1.0)

            hsT_t = hsT.rearrange("(kt p) t -> p kt t", p=P)

            # ---- Phase A: qkv projection (transposed layouts) ----
            for tb in range(NQB):
                hs_t = hs_pool.tile([P, KT, QB], f32r)
                nc.sync.dma_start(out=hs_t, in_=hsT_t[:, :, tb * QB:(tb + 1) * QB].bitcast(f32r))

                # qT / kT : [feat, token]
                for ft in range(FT_QK):
                    ps = mm_ps.tile([P, QB], f32, tag="mm")
                    for kt in range(KT):
                        nc.tensor.matmul(
                            ps,
                            (wqk_sb[:, kt, ft * P:(ft + 1) * P]),
                            (hs_t[:, kt, :]),
                            start=(kt == 0),
                            stop=(kt == KT - 1),
                        )
                    st = stage.tile([P, QB], f32r)
                    nc.scalar.activation(
                        out=st, in_=ps, func=AF.Identity,
                        bias=qkb_sb[:, ft:ft + 1], scale=1.0,
                    )
                    if ft < FT:
                        nc.sync.dma_start(
                            out=qT_dr[ft, :, tb * QB:(tb + 1) * QB].bitcast(f32r), in_=st)
                    else:
                        nc.sync.dma_start(
                            out=kT_dr[ft - FT, :, tb * QB:(tb + 1) * QB].bitcast(f32r), in_=st)

                # v : [token, feat] with ones column appended per head
                for tsub in range(QB // P):
                    tt = tb * (QB // P) + tsub
                    psv = mm_ps.tile([P, F], f32, tag="mm")
                    for kt in range(KT):
                        nc.tensor.matmul(
                            psv,
                            (hs_t[:, kt, tsub * P:(tsub + 1) * P]),
                            (wv_sb[:, kt, :]),
                            start=(kt == 0),
                            stop=(kt == KT - 1),
                        )
                    vt = vstage.tile([P, HG, D + 1], f32r)
                    nc.vector.tensor_add(
                        vt[:, :, 0:D],
                        psv.rearrange("p (h d) -> p h d", h=HG),
                        vb_sb.rearrange("p o (h d) -> p (o h) d", h=HG),
                    )
                    for hh in range(HG):
                        nc.vector.tensor_copy(vt[:, hh, D:D + 1], ones_col)
                    nc.sync.dma_start(
                        out=v_dr[:, tt, :, :].bitcast(f32r).rearrange("h p d -> p h d"), in_=vt)

            # ---- Phase B: attention per head ----
            for h in range(HG):
                hf = h // 2          # feat tile holding this head
                hr = (h % 2) * D     # row offset inside the feat tile
                vh = vh_pool.tile([P, TT, D + 1], f32r)
                nc.sync.dma_start(
                    out=vh, in_=v_dr[h, :, :, :].bitcast(f32r).rearrange("tt p d -> p tt d"))
                kh = kh_pool.tile([D, T], f32r)
                nc.sync.dma_start(out=kh, in_=kT_dr[hf, hr:hr + D, :].bitcast(f32r))

                for qb in range(NQB):
                    nkt = (qb + 1) * (QB // P)
                    qt = q_pool.tile([D, QB], f32r)
                    nc.sync.dma_start(
                        out=qt, in_=qT_dr[hf, hr:hr + D, qb * QB:(qb + 1) * QB].bitcast(f32r))

                    at = big.tile([P, NQB * (QB // P), QB], f32r, tag="big")
                    for kt in range(nkt):
                        ps = sc_ps.tile([P, QB], f32, tag="sc")
                        nc.tensor.matmul(
                            ps,
                            (kh[:, kt * P:(kt + 1) * P]),
                            (qt),
                            start=True, stop=True,
                        )
                        j = kt - qb * (QB // P)  # >=0 on diagonal tiles
                        if j < 0:
                            nc.scalar.activation(
                                out=at[:, kt, :], in_=ps, func=AF.Exp,
                                bias=actb_sb[:, h:h + 1], scale=acts_sb[:, h:h + 1],
                            )
                        else:
                            # Keep `at` ACT-only-written (the AV matmul can
                            # carry just one wait): ACT copies the diagonal
                            # 128 cols to SBUF, DVE adds the -1e30 mask
                            # there, ACT exps it back into `at`.
                            raw = raw_pool.tile([P, P], f32)
                            nc.scalar.activation(
                                out=raw, in_=ps[:, j * P:(j + 1) * P],
                                func=AF.Copy)
                            msk = msk_pool.tile([P, P], f32)
                            nc.vector.tensor_add(msk, raw, tri_sb)
                            if j > 0:
                                nc.scalar.activation(
                                    out=at[:, kt, 0:j * P], in_=ps[:, 0:j * P],
                                    func=AF.Copy, scale=0.0)
                            nc.scalar.activation(
                                out=at[:, kt, j * P:(j + 1) * P], in_=msk,
                                func=AF.Exp,
                                bias=actb_sb[:, h:h + 1], scale=acts_sb[:, h:h + 1],
                            )
                            if j < 3:
                                nc.scalar.activation(
                                    out=at[:, kt, (j + 1) * P:QB],
                                    in_=ps[:, (j + 1) * P:QB],
                                    func=AF.Exp,
                                    bias=actb_sb[:, h:h + 1],
                                    scale=acts_sb[:, h:h + 1],
                                )

                    avp = av_ps.tile([D + 1, QB], f32, tag="av")
                    for kt in range(nkt):
                        nc.tensor.matmul(
                            avp,
                            (vh[:, kt, :]),
                            (at[:, kt, :]),
                            start=(kt == 0),
                            stop=(kt == nkt - 1),
                        )

                    # normalize rows 0..63 by row 64 (the exp-sum)
                    rc = rc_pool.tile([1, QB], f32)
                    nc.vector.reciprocal(rc, avp[D:D + 1, :])
                    nc.sync.dma_start(out=rc_dr[h, qb], in_=rc)
                    bc = bc_pool.tile([D, QB], f32)
                    nc.sync.dma_start(
                        out=bc, in_=rc_dr[h, qb].partition_broadcast(D).rearrange(
                            "p o q -> p (o q)"))
                    ao = ao_pool.tile([D, QB], f32)
                    nc.vector.tensor_mul(ao, avp[0:D, :], bc)
                    nc.sync.dma_start(
                        out=ao_dr[hf, hr:hr + D, qb * QB:(qb + 1) * QB], in_=ao)

            # ---- Phase C: c_proj (partial, + bproj/2) ----
            for tt in range(TT):
                ot = out_pool.tile([P, E], f32)
                for eb in range(NEB):
                    ps = mm_ps.tile([P, EB], f32, tag="mm")
                    for ft in range(FT):
                        lt = aol_pool.tile([P, P], f32r)
                        nc.sync.dma_start(
                            out=lt, in_=ao_dr[ft, :, tt * P:(tt + 1) * P].bitcast(f32r))
                        nc.tensor.matmul(
                            ps,
                            (lt),
                            (wp_sb[:, ft, eb * EB:(eb + 1) * EB]),
                            start=(ft == 0),
                            stop=(ft == FT - 1),
                        )
                    nc.vector.tensor_add(
                        ot[:, eb * EB:(eb + 1) * EB], ps,
                        bp_sb[:, 0, eb * EB:(eb + 1) * EB],
                    )
                nc.sync.dma_start(out=out[tt], in_=ot)

    nc.compile()
    return nc


def make_in_maps(hidden_states, Wqkv, bqkv, Wproj, bproj, splat_scale, splat_bias):
    hs = np.asarray(hidden_states, dtype=np.float32)
    Wqkv = np.asarray(Wqkv, dtype=np.float32)
    bqkv = np.asarray(bqkv, dtype=np.float32)
    Wproj = np.asarray(Wproj, dtype=np.float32)
    bproj = np.asarray(bproj, dtype=np.float32)
    s = (1.0 + 0.01 * np.tanh(np.asarray(splat_scale, dtype=np.float32))).astype(np.float32)
    bsp = (0.001 * np.tanh(np.asarray(splat_bias, dtype=np.float32).reshape(H))).astype(np.float32)
    scale_factor = np.float32(1.0 / math.sqrt(D))

    Wq, Wk, Wv = Wqkv[0:E], Wqkv[E:2 * E], Wqkv[2 * E:3 * E]
    bq, bk, bv = bqkv[0:E], bqkv[E:2 * E], bqkv[2 * E:3 * E]

    tri = np.where(np.arange(P)[None, :] >= np.arange(P)[:, None],
                   np.float32(0.0), np.float32(-1e30)).astype(np.float32)

    group_maps = []
    for g in range(2):
        gs = slice(g * F, (g + 1) * F)
        wqkT = np.ascontiguousarray(
            np.concatenate([Wq[gs], Wk[gs]], axis=0).T).astype(np.float32)
        wvT = np.ascontiguousarray(Wv[gs].T).astype(np.float32)
        qk_bias = np.ascontiguousarray(
            np.concatenate([bq[gs], bk[gs]]).reshape(FT_QK, P).T).astype(np.float32)
        v_bias = np.ascontiguousarray(bv[gs].reshape(1, F)).astype(np.float32)
        wpT = np.ascontiguousarray(Wproj[:, gs].T).astype(np.float32)
        bp = (bproj * 0.5).reshape(1, E).astype(np.float32)
        hsl = slice(g * HG, (g + 1) * HG)
        act_s = np.tile((s[hsl] * scale_factor).reshape(1, HG), (P, 1)).astype(np.float32)
        act_b = np.tile(bsp[hsl].reshape(1, HG), (P, 1)).astype(np.float32)
        group_maps.append(dict(
            wqkT=wqkT, wvT=wvT, qk_bias=qk_bias, v_bias=v_bias,
            wpT=wpT, bp_half=bp, tri=tri, act_s=act_s, act_b=act_b,
        ))

    in_maps = []
    for c in range(8):
        b, g = c // 2, c % 2
        m = dict(group_maps[g])
        m["hsT"] = np.ascontiguousarray(hs[b].T).astype(np.float32)
        in_maps.append(m)
    return in_maps


def kernel(hidden_states, Wqkv, bqkv, Wproj, bproj, splat_scale, splat_bias,
           **run_kwargs):
    in_maps = make_in_maps(hidden_states, Wqkv, bqkv, Wproj, bproj,
                           splat_scale, splat_bias)
    nc = build_program()
    res = run_bass_kernel_spmd(nc, in_maps, core_ids=list(range(8)), **run_kwargs)
    outs = [np.asarray(r["out"], dtype=np.float32).reshape(T, E) for r in res.results]
    full = np.stack([outs[2 * b] + outs[2 * b + 1] for b in range(B)], axis=0)
    return full



# revision 20
# speedup vs baseline: 1.4823x; 1.4823x over previous
"""Trainium2 Bass kernel for ConservativeGSAAttention.

Sharding: 8 cores = 4 batches x 2 head-groups (8 heads each).
Each core computes qkv-proj + attention + its half of c_proj for one batch;
the host sums the two partial c_proj outputs per batch (the "all-reduce").

v2 design (vs the DRAM-scratch baseline):
  - Everything SBUF-resident in bf16 (weights, hs, q/k, v, attention
    probs, ao); PSUM accumulation stays fp32.
  - Causal masking via a DVE in-place tri-add on the PSUM diagonal block
    followed by a single ACT exp per score *pair* (two 512-wide blocks in
    one PSUM [128,1024] tile) -> far fewer ACT instructions.
  - Fully-masked columns are skipped in both the scores and AV matmuls
    (PSUM has_written semantics make partial-column accumulation work).
  - softmax denominator from an appended ones-column in v (AV row 64);
    reciprocal on DVE, partition-broadcast via a K=1 PE matmul, then one
    DVE multiply normalizes the 64-row AV output.
  - Phases are interleaved: qkv-projection (A) and c_proj (C) matmul
    "chunks" are woven between attention score/AV groups so the PE never
    idles long enough for HAM to re-throttle the clock.
"""

import math
from collections import deque
from contextlib import ExitStack

import numpy as np
import ml_dtypes

import concourse.bass as bass
import concourse.tile as tile
from concourse import bacc
from concourse import mybir
from concourse.bass_utils import run_bass_kernel_spmd

B, T, E, H, D = 4, 2048, 1024, 16, 64
HG = 8              # heads per core
F = HG * D          # 512 feats per group (for each of q, k, v)
P = 128
KT = E // P         # 8 contraction tiles for projections
TT = T // P         # 16 token tiles
QB = 512            # query block width
NQB = T // QB       # 4
EB = 512            # c_proj output block width

f32 = mybir.dt.float32
f32r = mybir.dt.float32r
bf16 = mybir.dt.bfloat16

AF = mybir.ActivationFunctionType


def build_program():
    nc = bacc.Bacc("TRN2", target_bir_lowering=False, debug=False)

    # ---- I/O ----
    hsT = nc.dram_tensor("hsT", [E, T], bf16, kind="ExternalInput").ap()
    wqkT = nc.dram_tensor("wqkT", [E, 2 * F], bf16, kind="ExternalInput").ap()
    wvT = nc.dram_tensor("wvT", [E, F], bf16, kind="ExternalInput").ap()
    wpT = nc.dram_tensor("wpT", [F, E], bf16, kind="ExternalInput").ap()
    qk_bias = nc.dram_tensor("qk_bias", [P, KT], f32, kind="ExternalInput").ap()
    v_bias = nc.dram_tensor("v_bias", [1, F], f32, kind="ExternalInput").ap()
    bp_half = nc.dram_tensor("bp_half", [1, E], f32, kind="ExternalInput").ap()
    tri = nc.dram_tensor("tri", [P, P], f32, kind="ExternalInput").ap()
    act_s = nc.dram_tensor("act_s", [P, HG], f32, kind="ExternalInput").ap()
    act_b = nc.dram_tensor("act_b", [P, HG], f32, kind="ExternalInput").ap()
    out = nc.dram_tensor("out", [TT, P, E], f32, kind="ExternalOutput").ap()

    with tile.TileContext(nc) as tc, ExitStack() as ctx:
        def pool(name, bufs, space="SBUF"):
            return ctx.enter_context(tc.tile_pool(name=name, bufs=bufs, space=space))

        consts = pool("consts", 1)
        # bufs must cover the deepest full-pair count per head (6 at qb=3):
        # AV is emitted after all of a head's scores, so exp(pair p) must
        # never wait on an at tile still unconsumed by this head's AV.
        at_pool = pool("atp", 8)
        rcbs_pool = pool("rcbs", 2)
        rc_pool = pool("rc", 2)
        aos_pool = pool("aos", 2)
        ot_pool = pool("otp", 2)
        sc_ps = pool("sc_ps", 2, "PSUM")
        av_ps = pool("av_ps", 2, "PSUM")
        rcb_ps = pool("rcb_ps", 1, "PSUM")
        mm_ps = pool("mm_ps", 1, "PSUM")

        # ---- resident constants / weights ----
        wqk_sb = consts.tile([P, KT, 2 * F], bf16)
        nc.sync.dma_start(out=wqk_sb, in_=wqkT.rearrange("(kt p) f -> p kt f", p=P))
        wv_sb = consts.tile([P, KT, F], bf16)
        nc.sync.dma_start(out=wv_sb, in_=wvT.rearrange("(kt p) f -> p kt f", p=P))
        wp_sb = consts.tile([P, F // P, E], bf16)
        nc.sync.dma_start(out=wp_sb, in_=wpT.rearrange("(ft p) e -> p ft e", p=P))
        qkb_sb = consts.tile([P, KT], f32)
        nc.sync.dma_start(out=qkb_sb, in_=qk_bias)
        vb_sb = consts.tile([P, 1, F], f32)
        nc.sync.dma_start(out=vb_sb, in_=v_bias.partition_broadcast(P))
        bp_sb = consts.tile([P, 1, E], f32)
        nc.sync.dma_start(out=bp_sb, in_=bp_half.partition_broadcast(P))
        tri_sb = consts.tile([P, P], f32)
        nc.sync.dma_start(out=tri_sb, in_=tri)
        acts_sb = consts.tile([P, HG], f32)
        nc.sync.dma_start(out=acts_sb, in_=act_s)
        actb_sb = consts.tile([P, HG], f32)
        nc.sync.dma_start(out=actb_sb, in_=act_b)
        ones_sb = consts.tile([1, D], bf16)
        nc.vector.memset(ones_sb, 1.0)

        # hs per token super-block (separate tiles keep deps exact)
        hsT_r = hsT.rearrange("(kt p) t -> p kt t", p=P)
        hs_sb = []
        for tb in range(NQB):
            h_t = consts.tile([P, KT, QB], bf16, name=f"hs_sb{tb}")
            nc.sync.dma_start(out=h_t, in_=hsT_r[:, :, tb * QB:(tb + 1) * QB])
            hs_sb.append(h_t)

        # qk per token super-block: [P, 8, QB] (ft 0-3 = q, 4-7 = k)
        qk_sb = [consts.tile([P, 2 * (F // P), QB], bf16, name=f"qk_sb{tb}")
                 for tb in range(NQB)]
        # v per token super-block: [P, 4, HG, D+1] (token tiles x heads)
        v_sb = [consts.tile([P, NQB, HG, D + 1], bf16, name=f"v_sb{tb}")
                for tb in range(NQB)]
        for tb in range(NQB):
            nc.vector.memset(v_sb[tb][:, :, :, D:D + 1], 1.0)
        # ao per query super-block: [P, 4, QB] (ft x query cols)
        ao_sb = [consts.tile([P, F // P, QB], bf16, name=f"ao_sb{qb}")
                 for qb in range(NQB)]

        # Fixed tiles for the two diagonal score pairs of every head.
        # Masked (never-exp-written) column ranges are identical across
        # heads/query-blocks, so zero them once; AV then always reads the
        # full 512-wide blocks (uniform matmul group APs keep the BIR
        # verifier happy) and masked columns contribute exact zeros.
        atd = [consts.tile([P, 2 * QB], bf16, name=f"atd{i}") for i in range(2)]
        nc.vector.memset(atd[0][:, QB:QB + P], 0.0)              # j=1 masked cols
        nc.vector.memset(atd[1][:, 0:2 * P], 0.0)                # j=2 masked cols
        nc.vector.memset(atd[1][:, QB:QB + 3 * P], 0.0)          # j=3 masked cols

        # ---------------- phase work units ----------------

        def emit_a_unit(tb, u):
            """A(tb) unit u: u<8 -> q/k feat tile u; u>=8 -> v token tile."""
            if u < 8:
                ft = u
                ps = mm_ps.tile([P, QB], f32, tag="mm", name="ps_a")
                for kt in range(KT):
                    nc.tensor.matmul(
                        ps,
                        wqk_sb[:, kt, ft * P:(ft + 1) * P],
                        hs_sb[tb][:, kt, :],
                        start=(kt == 0), stop=(kt == KT - 1),
                    )
                nc.vector.tensor_scalar_add(
                    qk_sb[tb][:, ft, :], ps, qkb_sb[:, ft:ft + 1])
            else:
                tsub = u - 8
                ps = mm_ps.tile([P, F], f32, tag="mm", name="ps_v")
                for kt in range(KT):
                    nc.tensor.matmul(
                        ps,
                        hs_sb[tb][:, kt, tsub * P:(tsub + 1) * P],
                        wv_sb[:, kt, :],
                        start=(kt == 0), stop=(kt == KT - 1),
                    )
                nc.vector.tensor_add(
                    v_sb[tb][:, tsub, :, 0:D],
                    ps.rearrange("p (h d) -> p h d", h=HG),
                    vb_sb.rearrange("p o (h d) -> p (o h) d", h=HG),
                )

        ot_live = {}

        def emit_c_unit(qb, u):
            """C(qb) unit u: token tile tt = qb*4 + u//2, eb = u%2."""
            tsub, eb = u // 2, u % 2
            tt = qb * NQB + tsub
            if eb == 0:
                ot_live[tt] = ot_pool.tile([P, E], f32, tag="ot", name="ot")
            ot = ot_live[tt]
            ps = mm_ps.tile([P, EB], f32, tag="mm", name="ps_c")
            for ft in range(F // P):
                nc.tensor.matmul(
                    ps,
                    ao_sb[qb][:, ft, tsub * P:(tsub + 1) * P],
                    wp_sb[:, ft, eb * EB:(eb + 1) * EB],
                    start=(ft == 0), stop=(ft == F // P - 1),
                )
            nc.vector.tensor_add(
                ot[:, eb * EB:(eb + 1) * EB], ps, bp_sb[:, 0, eb * EB:(eb + 1) * EB])
            if eb == 1:
                nc.sync.dma_start(out=out[tt], in_=ot)
                del ot_live[tt]

        # chunk queue: units of other phases woven into attention
        chunks = deque()

        def emit_chunk():
            if chunks:
                kind, a, b_ = chunks.popleft()
                if kind == "A":
                    emit_a_unit(a, b_)
                else:
                    emit_c_unit(a, b_)

        def flush_chunks_for(tb):
            """Emit any remaining A(tb) units still queued (dep barrier)."""
            while chunks and chunks[0][0] == "A" and chunks[0][1] == tb:
                emit_chunk()

        # ---------------- attention ----------------

        pend = [None]  # deferred (h, avp, rc) for rcb+norm

        def emit_norm():
            if pend[0] is None:
                return
            h, qb, avp, rc = pend[0]
            pend[0] = None
            hf, hr = h // 2, (h % 2) * D
            rcb = rcb_ps.tile([D, QB], f32, tag="rcb", name="rcb")
            nc.tensor.matmul(
                rcb, ones_sb, rc,
                start=True, stop=True)
            # DVE may read only one non-scalar PSUM input -> stage rcb in SBUF
            rcbs = rcbs_pool.tile([D, QB], f32, tag="rcbs", name="rcbs")
            nc.vector.tensor_copy(rcbs, rcb)
            if hr == 0:
                nc.vector.tensor_mul(
                    ao_sb[qb][0:D, hf, :], avp[0:D, :], rcbs)
            else:
                aos = aos_pool.tile([D, QB], bf16, tag="aos", name="aos")
                nc.vector.tensor_mul(aos, avp[0:D, :], rcbs)
                nc.sync.dma_start(out=ao_sb[qb][D:P, hf, :], in_=aos)

        def emit_head(qb, h):
            hf, hr = h // 2, (h % 2) * D
            nkt = NQB * (qb + 1)
            kq = qk_sb  # alias
            # --- scores + exp, in pairs of 128-key blocks ---
            npair = nkt // 2
            ats = []
            for p2 in range(npair):
                diag_i = p2 - (npair - 2)  # 0/1 for the two diagonal pairs
                if diag_i < 0:
                    atp = at_pool.tile([P, 2 * QB], bf16, tag="at", name="atp")
                else:
                    atp = atd[diag_i]
                sc = sc_ps.tile([P, 2 * QB], f32, tag="sc", name="sc")
                widths = []
                for half in range(2):
                    kt = 2 * p2 + half
                    j = kt - qb * NQB
                    off = j * P if j > 0 else 0
                    W = QB - off
                    widths.append((kt, j, off, W))
                    nc.tensor.matmul(
                        sc[:, half * QB:half * QB + W],
                        kq[kt // NQB][hr:hr + D, NQB + hf, (kt % NQB) * P:(kt % NQB + 1) * P],
                        kq[qb][hr:hr + D, hf, off:QB],
                        start=True, stop=True,
                    )
                    if j >= 0:
                        nc.vector.tensor_add(
                            sc[:, half * QB:half * QB + P],
                            sc[:, half * QB:half * QB + P], tri_sb)
                ats.append(atp)
                if diag_i < 0:
                    nc.scalar.activation(
                        out=atp, in_=sc, func=AF.Exp,
                        bias=actb_sb[:, h:h + 1], scale=acts_sb[:, h:h + 1])
                else:
                    for half, (kt, j, off, W) in enumerate(widths):
                        nc.scalar.activation(
                            out=atp[:, half * QB + off:(half + 1) * QB],
                            in_=sc[:, half * QB:half * QB + W], func=AF.Exp,
                            bias=actb_sb[:, h:h + 1], scale=acts_sb[:, h:h + 1])
                if p2 % 2 == 1:
                    emit_chunk()
            # deferred normalization of previous head (lets DVE recip finish)
            emit_norm()
            # --- AV (full-width uniform accumulation group) ---
            avp = av_ps.tile([D + 1, QB], f32, tag="av", name="avp")
            for p2 in range(npair):
                atp = ats[p2]
                for half in range(2):
                    kt = 2 * p2 + half
                    nc.tensor.matmul(
                        avp,
                        v_sb[kt // NQB][:, kt % NQB, h, :],
                        atp[:, half * QB:(half + 1) * QB],
                        start=(kt == 0), stop=(kt == nkt - 1),
                    )
            # rc is bf16 so the ones-broadcast matmul reads it natively
            # (f32 would force quarter-rate fp32 matmul; f32r fails DVE ISA
            # checks). 0.4% bf16 error on the softmax denom is tolerable.
            rc = rc_pool.tile([1, QB], bf16, tag="rc", name="rc")
            with nc.allow_low_precision(reason="bf16 softmax denom within tolerance"):
                nc.vector.reciprocal(rc, avp[D:D + 1, :])
            pend[0] = (h, qb, avp, rc)

        # ---------------- emission schedule ----------------

        # Upfront: A(0) fully (B(0) needs it); A(1..3) + C(0..2) via chunks.
        for u in range(12):
            emit_a_unit(0, u)
        for tb in range(1, NQB):
            for u in range(12):
                chunks.append(("A", tb, u))

        for qb in range(NQB):
            if qb > 0:
                flush_chunks_for(qb)  # qk/v for this qb must be emitted
            for h in range(HG):
                emit_head(qb, h)
            emit_norm()  # tail head of this qb
            for u in range(2 * NQB):
                chunks.append(("C", qb, u))

        while chunks:
            emit_chunk()

    nc.compile()
    return nc


def make_in_maps(hidden_states, Wqkv, bqkv, Wproj, bproj, splat_scale, splat_bias):
    hs = np.asarray(hidden_states, dtype=np.float32)
    Wqkv = np.asarray(Wqkv, dtype=np.float32)
    bqkv = np.asarray(bqkv, dtype=np.float32)
    Wproj = np.asarray(Wproj, dtype=np.float32)
    bproj = np.asarray(bproj, dtype=np.float32)
    s = (1.0 + 0.01 * np.tanh(np.asarray(splat_scale, dtype=np.float32))).astype(np.float32)
    bsp = (0.001 * np.tanh(np.asarray(splat_bias, dtype=np.float32).reshape(H))).astype(np.float32)
    scale_factor = np.float32(1.0 / math.sqrt(D))

    Wq, Wk, Wv = Wqkv[0:E], Wqkv[E:2 * E], Wqkv[2 * E:3 * E]
    bq, bk, bv = bqkv[0:E], bqkv[E:2 * E], bqkv[2 * E:3 * E]

    tri = np.where(np.arange(P)[None, :] >= np.arange(P)[:, None],
                   np.float32(0.0), np.float32(-1e30)).astype(np.float32)

    bf = ml_dtypes.bfloat16
    group_maps = []
    for g in range(2):
        gs = slice(g * F, (g + 1) * F)
        wqkT = np.ascontiguousarray(
            np.concatenate([Wq[gs], Wk[gs]], axis=0).T).astype(bf)
        wvT = np.ascontiguousarray(Wv[gs].T).astype(bf)
        # qk_bias columns are the 8 feat tiles of [q(4), k(4)]
        qk_bias = np.ascontiguousarray(
            np.concatenate([bq[gs], bk[gs]]).reshape(KT, P).T).astype(np.float32)
        v_bias = np.ascontiguousarray(bv[gs].reshape(1, F)).astype(np.float32)
        wpT = np.ascontiguousarray(Wproj[:, gs].T).astype(bf)
        bp = (bproj * 0.5).reshape(1, E).astype(np.float32)
        hsl = slice(g * HG, (g + 1) * HG)
        act_s = np.tile((s[hsl] * scale_factor).reshape(1, HG), (P, 1)).astype(np.float32)
        act_b = np.tile(bsp[hsl].reshape(1, HG), (P, 1)).astype(np.float32)
        group_maps.append(dict(
            wqkT=wqkT, wvT=wvT, qk_bias=qk_bias, v_bias=v_bias,
            wpT=wpT, bp_half=bp, tri=tri, act_s=act_s, act_b=act_b,
        ))

    in_maps = []
    for c in range(8):
        b, g = c // 2, c % 2
        m = dict(group_maps[g])
        m["hsT"] = np.ascontiguousarray(hs[b].T).astype(bf)
        in_maps.append(m)
    return in_maps


def kernel(hidden_states, Wqkv, bqkv, Wproj, bproj, splat_scale, splat_bias,
           **run_kwargs):
    in_maps = make_in_maps(hidden_states, Wqkv, bqkv, Wproj, bproj,
                           splat_scale, splat_bias)
    nc = build_program()
    res = run_bass_kernel_spmd(nc, in_maps, core_ids=list(range(8)), **run_kwargs)
    outs = [np.asarray(r["out"], dtype=np.float32).reshape(T, E) for r in res.results]
    full = np.stack([outs[2 * b] + outs[2 * b + 1] for b in range(B)], axis=0)
    return full
